# revision 14
# baseline (speedup 1.0000x reference)
"""MemristorDense forward on 8 Trainium2 NeuronCores.

Math
----
Reference computes, with R = n_in+1 rows (x plus a ones bias-row), C = 2*n_out
interleaved pos/neg columns:

    y = 0.5 * sum_r sign(x) * (W+m9) * exp(L[b,r] * log2(n[r,c]))

with L = ln(max(2|x|,1e-12)), m9 = max_w/9.  Write n = 2^gbar * (1+v)
(gbar = midrange of log2(n), |v| <~ 0.26) and z = log2(2|x|).  Then
exp(L*log2 n) = (2|x|)^gbar * (1+v)^z, and the binomial series
(1+v)^z = sum_k C(z,k) v^k turns the [B,R,C] elementwise-pow contraction
into K+1 TensorEngine matmuls:

    y = sum_k  A_k.T @ W_k,   A_0 = x*|x|^(gbar-1)   (2^(gbar-1) folded into W)
                              A_1 = A_0 * z,  A_2 = A_1 * (z-1)/2
                              W_k = W_0 * v^k,  W_0 = +-(w + m9) * 2^(gbar-1)

K=2 suffices: truncation + fp16 quantization land at ~3e-3 relative against
the 2e-2 gate.  The ones bias-row is removed from the series: its
contribution 0.5*(b+m9)*n[n_in,c] is b-independent and exact, computed on the
host and folded into a k=0-only contraction chunk whose A-column is 0.5.

Everything on device is fp16; accumulation is fp32 in PSUM.  ScalarE does
only Ln and Exp (one act-table set, load hoisted before data arrives);
Square runs on VectorE.  DMAs are partition-split across two issue engines
because DMA time is packet-bound (~15ns per partition-packet), and dummy
matmuls warm the PE clock gate before the real accumulation.

Sharding: tensor-parallel over output columns (64 pos + 64 neg per core),
A-side replicated -- no collectives, gather is a pure concat.

Device layout: tiles are [128, 8*128] with tile[p, 128*ch + c] =
host_row[128*ch + p, c]; x-side free index is (ch, b), W-side is (ch, c).
"""

import numpy as np

import concourse.bacc as bacc
import concourse.tile as tile
import concourse.mybir as mybir
from concourse.bass_utils import run_bass_kernel_spmd

F32 = mybir.dt.float32
F16 = mybir.dt.float16
ALU = mybir.AluOpType
ACT = mybir.ActivationFunctionType

NCORES = 8
B = 128
N_IN = 1024
N_OUT = 512
NCH = 8                 # full 128-row chunks of real x rows
RC = NCH * 128          # 1024 real contraction rows
RP = RC + 128           # + bias chunk (k=0 only)
CS = N_OUT // NCORES    # 64 output columns per core
LN2 = 0.6931471805599453
NWARM = 26              # PE warm-up dummy matmuls

# Stashed by kernel() for the test harness (exec_time_ns, trace paths).
LAST_RESULTS = None

_ACT_SET = "natural_log_exp_and_others"
_ACT_SHARED = {
    ACT.Square, ACT.Ln, ACT.Exp, ACT.Copy, ACT.Identity, ACT.Abs, ACT.Sign,
    ACT.MemsetZero,
}


def _patched_tables(arch, _orig=bacc.get_activation_tables):
    """Steer the act-table-load pass to a single table set: every function we
    use (ln/exp/copy) lives in natural_log_exp_and_others, but the greedy
    per-instruction chooser would otherwise pick several sets (~1.3us
    ACT_TABLE_LOAD each on the critical ScalarE chain).  Set names and order
    are preserved so act_func_set_id stays a valid act_info.json index."""
    t = _orig(arch)
    return {
        name: (funcs if name == _ACT_SET else (funcs - _ACT_SHARED))
        for name, funcs in t.items()
    }


def _build_program(gbar: float):
    orig_tables = bacc.get_activation_tables
    bacc.get_activation_tables = _patched_tables
    try:
        return _build_program_inner(gbar)
    finally:
        bacc.get_activation_tables = orig_tables


def _build_program_inner(gbar: float):
    nc = bacc.Bacc(
        "TRN2", target_bir_lowering=False, debug=False, num_devices=NCORES
    )
    xt_d = nc.dram_tensor("xt_in", [128, RC], F16, kind="ExternalInput").ap()
    w_d = nc.dram_tensor("w_in", [128, RP], F16, kind="ExternalInput").ap()
    v_d = nc.dram_tensor("v_in", [128, RC], F16, kind="ExternalInput").ap()
    y_d = nc.dram_tensor("y_out", [B, CS], F32, kind="ExternalOutput").ap()

    with tile.TileContext(nc) as tc:
        with (
            tc.tile_pool(name="pers", bufs=1) as pool,
            tc.tile_pool(name="acc", bufs=1, space="PSUM") as pspool,
        ):
            eps = pool.tile([128, 1], F32)
            nc.vector.memset(eps[:], 1e-24)
            wsrc = pool.tile([128, 128], F16)
            nc.vector.memset(wsrc[:], 1.0)
            xT = pool.tile([128, RC], F16)
            Sq = pool.tile([128, RC], F16)
            Lr = pool.tile([128, RC], F16)
            E1 = pool.tile([128, RC], F16)
            Z = pool.tile([128, RC], F16)
            Z1h = pool.tile([128, RC], F16)
            A0 = pool.tile([128, RP], F16)
            A1 = pool.tile([128, RC], F16)
            A2 = pool.tile([128, RC], F16)
            W0 = pool.tile([128, RP], F16)
            W1 = pool.tile([128, RC], F16)
            W2 = pool.tile([128, RC], F16)
            vt = pool.tile([128, RC], F16)
            yneg = pool.tile([128, CS], F32)
            ysb = pool.tile([128, CS], F32)
            acc = pspool.tile([128, 2 * CS], F32)
            warm = pspool.tile([128, 128], F32)

            # bias chunk of A0: 0.5 on partition 0, zero elsewhere
            nc.vector.memset(A0[:, RC:RP], 0.0)
            nc.vector.memset(A0[0:1, RC:RP], 0.5)

            # PE warm-up: dummies depend only on the wsrc memset, so they
            # start right after the preamble and hold HAM at K=8/8 until the
            # real matmuls arrive.
            for _ in range(NWARM):
                nc.tensor.matmul(warm[:], wsrc[:], wsrc[:], start=True, stop=True)

            # Input DMA, partition-split.  sync + scalar drive the fast
            # hardware DGE queues (x and W); gpsimd's software DGE is ~3x
            # slower and gets v, which is needed latest (only by W1).
            nc.sync.dma_start(xT[0:64, :], xt_d[0:64, :])
            nc.scalar.dma_start(xT[64:128, :], xt_d[64:128, :])
            nc.sync.dma_start(W0[0:64, :], w_d[0:64, :])
            nc.scalar.dma_start(W0[64:128, :], w_d[64:128, :])
            nc.gpsimd.dma_start(vt[0:64, :], v_d[0:64, :])
            nc.gpsimd.dma_start(vt[64:128, :], v_d[64:128, :])

            # Uneven column halves: the second (last) slice is small so the
            # trailing Exp -> A0 -> A1 -> A2 -> matmul chain is short.
            H = 640
            halves = [slice(0, H), slice(H, RC)]
            # x chain: Sq = x^2 on DVE; Lr = ln(Sq+eps) on ScalarE (= 2 ln|x|);
            # E1 = |x|^(gbar-1) on ScalarE.  Ln/Exp interleave freely --
            # both live in the same act-table set.
            for sl in halves:
                nc.vector.tensor_mul(Sq[:, sl], xT[:, sl], xT[:, sl])
            for sl in halves:
                nc.scalar.activation(Lr[:, sl], Sq[:, sl], ACT.Ln, bias=eps[:])
                nc.scalar.activation(
                    E1[:, sl], Lr[:, sl], ACT.Exp, scale=(gbar - 1.0) / 2.0
                )

            # DVE: z = Lr/(2 ln2) + 1 ; (z-1)/2 = Lr/(4 ln2) ;
            # A0 = x*E1 ; A1 = A0*z ; A2 = A1*(z-1)/2
            for sl in halves:
                nc.vector.tensor_scalar(
                    Z[:, sl], Lr[:, sl], 1.0 / (2 * LN2), 1.0, ALU.mult, ALU.add
                )
                nc.vector.tensor_scalar(
                    Z1h[:, sl], Lr[:, sl], 1.0 / (4 * LN2), None, ALU.mult
                )
                nc.vector.tensor_mul(A0[:, sl], xT[:, sl], E1[:, sl])
                nc.vector.tensor_mul(A1[:, sl], A0[:, sl], Z[:, sl])
                nc.vector.tensor_mul(A2[:, sl], A1[:, sl], Z1h[:, sl])

            # W chain (ready as soon as DMA lands; overlaps ScalarE work)
            for sl in halves:
                nc.vector.tensor_mul(W1[:, sl], W0[:, sl], vt[:, sl])
                nc.vector.tensor_mul(W2[:, sl], W1[:, sl], vt[:, sl])

            # 25 matmuls, one PSUM accumulation group.
            # k=0: 8 x-chunks + bias chunk ; k=1,2: 8 x-chunks each.
            nc.tensor.matmul(
                acc[:], A0[:, RC:RP], W0[:, RC:RP], start=True, stop=False
            )
            for k, (Ak, Wk) in enumerate(((A0, W0), (A1, W1), (A2, W2))):
                for ch in range(NCH):
                    sl = slice(ch * 128, (ch + 1) * 128)
                    nc.tensor.matmul(
                        acc[:], Ak[:, sl], Wk[:, sl],
                        start=False,
                        stop=(k == 2 and ch == NCH - 1),
                    )

            # y = pos + (-neg); neg columns were pre-negated on host
            nc.scalar.copy(yneg[:], acc[:, CS : 2 * CS])
            nc.vector.tensor_add(ysb[:], acc[:, 0:CS], yneg[:])
            nc.sync.dma_start(y_d[0:64, :], ysb[0:64, :])
            nc.scalar.dma_start(y_d[64:128, :], ysb[64:128, :])

    nc.compile()
    return nc


def _shard_inputs(x, w_pos, w_neg, b_pos, b_neg, n_param, m9, gbar):
    """Per-core input maps: slicing, layout swizzles, dtype casts, and affine
    weight prep (w -> +-(w+m9)*2^(gbar-1), n -> n*2^-gbar - 1, bias fold)."""

    def swizzle(host):  # [nch*128, 128] -> [128, nch*128] device layout
        nch = host.shape[0] // 128
        return np.ascontiguousarray(
            host.reshape(nch, 128, 128).transpose(1, 0, 2).reshape(128, nch * 128)
        )

    # xT[p, 128*ch + b] = x[b, 128*ch + p]
    xT = np.ascontiguousarray(
        x.astype(np.float16).reshape(128, NCH, 128).transpose(2, 1, 0).reshape(128, RC)
    )

    # exact bias-row fold: contribution 0.5*(b+m9)*n[N_IN, c]  (A-col is 0.5)
    nb = n_param[N_IN, :].astype(np.float64)
    yb_pos = (b_pos.astype(np.float64) + m9) * nb[0::2]
    yb_neg = -(b_neg.astype(np.float64) + m9) * nb[1::2]

    wsc = np.float64(2.0) ** (gbar - 1.0)  # folded A-side constant
    scale_v = np.float64(2.0) ** (-gbar)
    in_maps = []
    for j in range(NCORES):
        cp = slice(CS * j, CS * (j + 1))
        W_host = np.zeros((RP, 128), np.float16)
        W_host[:N_IN, 0:CS] = (w_pos[:, cp] + np.float32(m9)) * wsc
        W_host[:N_IN, CS:128] = -(w_neg[:, cp] + np.float32(m9)) * wsc
        W_host[N_IN, 0:CS] = yb_pos[cp]
        W_host[N_IN, CS:128] = yb_neg[cp]
        V_host = np.empty((RC, 128), np.float16)
        V_host[:, 0:CS] = n_param[:N_IN, 2 * CS * j : 2 * CS * (j + 1) : 2] * scale_v - 1.0
        V_host[:, CS:128] = (
            n_param[:N_IN, 2 * CS * j + 1 : 2 * CS * (j + 1) : 2] * scale_v - 1.0
        )
        in_maps.append(
            {
                "xt_in": xT,
                "w_in": swizzle(W_host),
                "v_in": swizzle(V_host),
            }
        )
    return in_maps


def kernel(x, w_pos, w_neg, b_pos, b_neg, n_param, **run_kwargs):
    global LAST_RESULTS
    x = np.ascontiguousarray(np.asarray(x, np.float32))
    w_pos = np.asarray(w_pos, np.float32)
    w_neg = np.asarray(w_neg, np.float32)
    b_pos = np.asarray(b_pos, np.float32)
    b_neg = np.asarray(b_neg, np.float32)
    n_param = np.asarray(n_param, np.float32)

    max_w = float(max(w_pos.max(), w_neg.max(), b_pos.max(), b_neg.max()))
    m9 = max_w / 9.0
    gbar = float(
        0.5 * (np.log2(float(n_param.min())) + np.log2(float(n_param.max())))
    )

    nc = _build_program(gbar)
    in_maps = _shard_inputs(x, w_pos, w_neg, b_pos, b_neg, n_param, m9, gbar)
    res = run_bass_kernel_spmd(nc, in_maps, list(range(NCORES)), **run_kwargs)
    LAST_RESULTS = res
    return np.concatenate([res.results[j]["y_out"] for j in range(NCORES)], axis=1)


# revision 19
# speedup vs baseline: 1.1403x; 1.1403x over previous
"""MemristorDense forward on 8 Trainium2 NeuronCores.

Math
----
Reference computes, with R = n_in+1 rows (x plus a ones bias-row), C = 2*n_out
interleaved pos/neg columns:

    y = 0.5 * sum_r sign(x) * (W+m9) * exp(L[b,r] * log2(n[r,c]))

with L = ln(max(2|x|,1e-12)), m9 = max_w/9.  Write n = 2^gbar * (1+v)
(gbar = midrange of log2(n), |v| <~ 0.26) and z = log2(2|x|).  Then
exp(L*log2 n) = (2|x|)^gbar * (1+v)^z, and the binomial series
(1+v)^z = sum_k C(z,k) v^k turns the [B,R,C] elementwise-pow contraction
into K+1 TensorEngine matmuls:

    y = sum_k  A_k.T @ W_k,   A_0 = x*|x|^(gbar-1)   (2^(gbar-1) folded into W)
                              A_1 = A_0 * z,  A_2 = A_1 * (z-1)/2
                              W_k = W_0 * v^k,  W_0 = +-(w + m9) * 2^(gbar-1)

K=2 suffices: truncation + fp16 quantization land at ~3e-3 relative against
the 2e-2 gate.  The ones bias-row is removed from the series: its
contribution 0.5*(b+m9)*n[n_in,c] is b-independent and exact, computed on the
host and folded into a k=0-only contraction chunk whose A-column is 0.5.

Everything on device is fp16; accumulation is fp32 in PSUM.  ScalarE does
only Ln and Exp (one act-table set, load hoisted before data arrives);
Square runs on VectorE.  DMAs are partition-split across two issue engines
because DMA time is packet-bound (~15ns per partition-packet), and dummy
matmuls warm the PE clock gate before the real accumulation.

Sharding: tensor-parallel over output columns (64 pos + 64 neg per core),
A-side replicated -- no collectives, gather is a pure concat.

Device layout: tiles are [128, 8*128] with tile[p, 128*ch + c] =
host_row[128*ch + p, c]; x-side free index is (ch, b), W-side is (ch, c).
"""

import numpy as np

import concourse.bacc as bacc
import concourse.tile as tile
import concourse.mybir as mybir
from concourse.bass_utils import run_bass_kernel_spmd

F32 = mybir.dt.float32
F16 = mybir.dt.float16
ALU = mybir.AluOpType
ACT = mybir.ActivationFunctionType

NCORES = 8
B = 128
N_IN = 1024
N_OUT = 512
NCH = 8                 # full 128-row chunks of real x rows
RC = NCH * 128          # 1024 real contraction rows
RP = RC + 128           # + bias chunk (k=0 only)
CS = N_OUT // NCORES    # 64 output columns per core
LN2 = 0.6931471805599453
# Stashed by kernel() for the test harness (exec_time_ns, trace paths).
LAST_RESULTS = None

_ACT_SET = "natural_log_exp_and_others"
_ACT_SHARED = {
    ACT.Square, ACT.Ln, ACT.Exp, ACT.Copy, ACT.Identity, ACT.Abs, ACT.Sign,
    ACT.MemsetZero,
}


def _patched_tables(arch, _orig=bacc.get_activation_tables):
    """Steer the act-table-load pass to a single table set: every function we
    use (ln/exp/copy) lives in natural_log_exp_and_others, but the greedy
    per-instruction chooser would otherwise pick several sets (~1.3us
    ACT_TABLE_LOAD each on the critical ScalarE chain).  Set names and order
    are preserved so act_func_set_id stays a valid act_info.json index."""
    t = _orig(arch)
    return {
        name: (funcs if name == _ACT_SET else (funcs - _ACT_SHARED))
        for name, funcs in t.items()
    }


def _build_program(gbar: float):
    orig_tables = bacc.get_activation_tables
    bacc.get_activation_tables = _patched_tables
    try:
        return _build_program_inner(gbar)
    finally:
        bacc.get_activation_tables = orig_tables


def _build_program_inner(gbar: float):
    nc = bacc.Bacc(
        "TRN2", target_bir_lowering=False, debug=False, num_devices=NCORES
    )
    xt_d = nc.dram_tensor("xt_in", [128, RC], F16, kind="ExternalInput").ap()
    w_d = nc.dram_tensor("w_in", [128, RP], F16, kind="ExternalInput").ap()
    v_d = nc.dram_tensor("v_in", [128, RC], F16, kind="ExternalInput").ap()
    y_d = nc.dram_tensor("y_out", [B, CS], F32, kind="ExternalOutput").ap()

    with tile.TileContext(nc) as tc:
        with (
            tc.tile_pool(name="pers", bufs=1) as pool,
            tc.tile_pool(name="acc", bufs=1, space="PSUM") as pspool,
        ):
            eps = pool.tile([128, 1], F32)
            nc.vector.memset(eps[:], 1e-24)
            xT = pool.tile([128, RC], F16)
            Ax = pool.tile([128, RC], F16)
            Lr = pool.tile([128, RC], F16)
            E1 = pool.tile([128, RC], F16)
            Z = pool.tile([128, RC], F16)
            Z1h = pool.tile([128, RC], F16)
            A0 = pool.tile([128, RP], F16)
            A1 = pool.tile([128, RC], F16)
            A2 = pool.tile([128, RC], F16)
            W0 = pool.tile([128, RP], F16)
            W1 = pool.tile([128, RC], F16)
            W2 = pool.tile([128, RC], F16)
            vt = pool.tile([128, RC], F16)
            yneg = pool.tile([128, CS], F32)
            ysb = pool.tile([128, CS], F32)
            acc = pspool.tile([128, 2 * CS], F32)

            # bias chunk of A0: 0.5 on partition 0, zero elsewhere
            nc.vector.memset(A0[:, RC:RP], 0.0)
            nc.vector.memset(A0[0:1, RC:RP], 0.5)

            # Input DMA: two parallel streams (sync = HWDGE, gpsimd = SWDGE),
            # partition-split, ordered x -> W -> v per queue so x (which heads
            # the serial ScalarE chain) completes first.  ScalarE issues no
            # input DMA: its queue starts with the act-table load.
            nc.sync.dma_start(xT[0:64, :], xt_d[0:64, :])
            nc.gpsimd.dma_start(xT[64:128, :], xt_d[64:128, :])
            nc.sync.dma_start(W0[0:64, :], w_d[0:64, :])
            nc.gpsimd.dma_start(W0[64:128, :], w_d[64:128, :])
            nc.sync.dma_start(vt[0:64, :], v_d[0:64, :])
            nc.gpsimd.dma_start(vt[64:128, :], v_d[64:128, :])

            # Uneven column halves: the second (last) slice is small so the
            # trailing Exp -> A0 -> A1 -> A2 -> matmul chain is short.
            H = 640
            halves = [slice(0, H), slice(H, RC)]
            # x chain: Ax = x^2 on DVE; Lr = ln(4x^2+eps) = 2 ln(2|x|) on
            # ScalarE (the 4x folds into the Ln input scale); E1 =
            # (2|x|)^(gbar-1) on ScalarE.  Ln/Exp interleave freely -- both
            # live in the same act-table set.
            for sl in halves:
                nc.vector.tensor_mul(Ax[:, sl], xT[:, sl], xT[:, sl])
            for sl in halves:
                nc.scalar.activation(
                    Lr[:, sl], Ax[:, sl], ACT.Ln, bias=eps[:], scale=4.0
                )
                nc.scalar.activation(
                    E1[:, sl], Lr[:, sl], ACT.Exp, scale=(gbar - 1.0) / 2.0
                )

            # DVE: z = Lr/(2 ln2) ; (z-1)/2 = Lr/(4 ln2) - 1/2 ;
            # A0 = x*E1 ; A1 = A0*z ; A2 = A1*(z-1)/2.
            # A-chain strictly before the W-chain in the DVE FIFO: W1/W2 wait
            # on the late v DMA and must not head-of-line-block the A path.
            for sl in halves:
                nc.vector.tensor_scalar(
                    Z[:, sl], Lr[:, sl], 1.0 / (2 * LN2), None, ALU.mult
                )
                nc.vector.tensor_scalar(
                    Z1h[:, sl], Lr[:, sl], 1.0 / (4 * LN2), -0.5, ALU.mult, ALU.add
                )
                nc.vector.tensor_mul(A0[:, sl], xT[:, sl], E1[:, sl])
                nc.vector.tensor_mul(A1[:, sl], A0[:, sl], Z[:, sl])
                nc.vector.tensor_mul(A2[:, sl], A1[:, sl], Z1h[:, sl])

            # W chain
            for sl in halves:
                nc.vector.tensor_mul(W1[:, sl], W0[:, sl], vt[:, sl])
                nc.vector.tensor_mul(W2[:, sl], W1[:, sl], vt[:, sl])

            # 25 matmuls, one PSUM accumulation group.
            # k=0: 8 x-chunks + bias chunk ; k=1,2: 8 x-chunks each.
            nc.tensor.matmul(
                acc[:], A0[:, RC:RP], W0[:, RC:RP], start=True, stop=False
            )
            for k, (Ak, Wk) in enumerate(((A0, W0), (A1, W1), (A2, W2))):
                for ch in range(NCH):
                    sl = slice(ch * 128, (ch + 1) * 128)
                    nc.tensor.matmul(
                        acc[:], Ak[:, sl], Wk[:, sl],
                        start=False,
                        stop=(k == 2 and ch == NCH - 1),
                    )

            # y = pos + (-neg); neg columns were pre-negated on host
            nc.scalar.copy(yneg[:], acc[:, CS : 2 * CS])
            nc.vector.tensor_add(ysb[:], acc[:, 0:CS], yneg[:])
            nc.sync.dma_start(y_d[0:64, :], ysb[0:64, :])
            nc.scalar.dma_start(y_d[64:128, :], ysb[64:128, :])

    nc.compile()
    return nc


def _shard_inputs(x, w_pos, w_neg, b_pos, b_neg, n_param, m9, gbar):
    """Per-core input maps: slicing, layout swizzles, dtype casts, and affine
    weight prep (w -> +-(w+m9)*2^(gbar-1), n -> n*2^-gbar - 1, bias fold)."""

    def swizzle(host):  # [nch*128, 128] -> [128, nch*128] device layout
        nch = host.shape[0] // 128
        return np.ascontiguousarray(
            host.reshape(nch, 128, 128).transpose(1, 0, 2).reshape(128, nch * 128)
        )

    # xT[p, 128*ch + b] = x[b, 128*ch + p]
    xT = np.ascontiguousarray(
        x.astype(np.float16).reshape(128, NCH, 128).transpose(2, 1, 0).reshape(128, RC)
    )

    # exact bias-row fold: contribution 0.5*(b+m9)*n[N_IN, c]  (A-col is 0.5)
    nb = n_param[N_IN, :].astype(np.float64)
    yb_pos = (b_pos.astype(np.float64) + m9) * nb[0::2]
    yb_neg = -(b_neg.astype(np.float64) + m9) * nb[1::2]

    scale_v = np.float64(2.0) ** (-gbar)
    in_maps = []
    for j in range(NCORES):
        cp = slice(CS * j, CS * (j + 1))
        W_host = np.zeros((RP, 128), np.float16)
        W_host[:N_IN, 0:CS] = w_pos[:, cp] + np.float32(m9)
        W_host[:N_IN, CS:128] = -(w_neg[:, cp] + np.float32(m9))
        W_host[N_IN, 0:CS] = yb_pos[cp]
        W_host[N_IN, CS:128] = yb_neg[cp]
        V_host = np.empty((RC, 128), np.float16)
        V_host[:, 0:CS] = n_param[:N_IN, 2 * CS * j : 2 * CS * (j + 1) : 2] * scale_v - 1.0
        V_host[:, CS:128] = (
            n_param[:N_IN, 2 * CS * j + 1 : 2 * CS * (j + 1) : 2] * scale_v - 1.0
        )
        in_maps.append(
            {
                "xt_in": xT,
                "w_in": swizzle(W_host),
                "v_in": swizzle(V_host),
            }
        )
    return in_maps


def kernel(x, w_pos, w_neg, b_pos, b_neg, n_param, **run_kwargs):
    global LAST_RESULTS
    x = np.ascontiguousarray(np.asarray(x, np.float32))
    w_pos = np.asarray(w_pos, np.float32)
    w_neg = np.asarray(w_neg, np.float32)
    b_pos = np.asarray(b_pos, np.float32)
    b_neg = np.asarray(b_neg, np.float32)
    n_param = np.asarray(n_param, np.float32)

    max_w = float(max(w_pos.max(), w_neg.max(), b_pos.max(), b_neg.max()))
    m9 = max_w / 9.0
    gbar = float(
        0.5 * (np.log2(float(n_param.min())) + np.log2(float(n_param.max())))
    )

    nc = _build_program(gbar)
    in_maps = _shard_inputs(x, w_pos, w_neg, b_pos, b_neg, n_param, m9, gbar)
    res = run_bass_kernel_spmd(nc, in_maps, list(range(NCORES)), **run_kwargs)
    LAST_RESULTS = res
    return np.concatenate([res.results[j]["y_out"] for j in range(NCORES)], axis=1)


# revision 21
# speedup vs baseline: 1.1508x; 1.0092x over previous
"""MemristorDense forward on 8 Trainium2 NeuronCores.

Math
----
Reference computes, with R = n_in+1 rows (x plus a ones bias-row), C = 2*n_out
interleaved pos/neg columns:

    y = 0.5 * sum_r sign(x) * (W+m9) * exp(L[b,r] * log2(n[r,c]))

with L = ln(max(2|x|,1e-12)), m9 = max_w/9.  Write n = 2^gbar * (1+v)
(gbar = midrange of log2(n), |v| <~ 0.26) and z = log2(2|x|).  Then
exp(L*log2 n) = (2|x|)^gbar * (1+v)^z, and the binomial series
(1+v)^z = sum_k C(z,k) v^k turns the [B,R,C] elementwise-pow contraction
into K+1 TensorEngine matmuls:

    y = sum_k  A_k.T @ W_k,   A_0 = x*|x|^(gbar-1)   (2^(gbar-1) folded into W)
                              A_1 = A_0 * z,  A_2 = A_1 * (z-1)/2
                              W_k = W_0 * v^k,  W_0 = +-(w + m9) * 2^(gbar-1)

K=2 suffices: truncation + fp16 quantization land at ~3e-3 relative against
the 2e-2 gate.  The ones bias-row is removed from the series: its
contribution 0.5*(b+m9)*n[n_in,c] is b-independent and exact, computed on the
host and folded into a k=0-only contraction chunk whose A-column is 0.5.

Everything on device is fp16; accumulation is fp32 in PSUM.  ScalarE does
only Ln and Exp (one act-table set, load hoisted before data arrives);
Square runs on VectorE.  DMAs are partition-split across two issue engines
because DMA time is packet-bound (~15ns per partition-packet), and dummy
matmuls warm the PE clock gate before the real accumulation.

Sharding: tensor-parallel over output columns (64 pos + 64 neg per core),
A-side replicated -- no collectives, gather is a pure concat.

Device layout: tiles are [128, 8*128] with tile[p, 128*ch + c] =
host_row[128*ch + p, c]; x-side free index is (ch, b), W-side is (ch, c).
"""

import numpy as np

import concourse.bacc as bacc
import concourse.tile as tile
import concourse.mybir as mybir
from concourse.bass_utils import run_bass_kernel_spmd

F32 = mybir.dt.float32
F16 = mybir.dt.float16
ALU = mybir.AluOpType
ACT = mybir.ActivationFunctionType

NCORES = 8
B = 128
N_IN = 1024
N_OUT = 512
NCH = 8                 # full 128-row chunks of real x rows
RC = NCH * 128          # 1024 real contraction rows
RP = RC + 128           # + bias chunk (k=0 only)
CS = N_OUT // NCORES    # 64 output columns per core
LN2 = 0.6931471805599453
# Stashed by kernel() for the test harness (exec_time_ns, trace paths).
LAST_RESULTS = None

_ACT_SET = "natural_log_exp_and_others"
_ACT_SHARED = {
    ACT.Square, ACT.Ln, ACT.Exp, ACT.Copy, ACT.Identity, ACT.Abs, ACT.Sign,
    ACT.MemsetZero,
}


def _patched_tables(arch, _orig=bacc.get_activation_tables):
    """Steer the act-table-load pass to a single table set: every function we
    use (ln/exp/copy) lives in natural_log_exp_and_others, but the greedy
    per-instruction chooser would otherwise pick several sets (~1.3us
    ACT_TABLE_LOAD each on the critical ScalarE chain).  Set names and order
    are preserved so act_func_set_id stays a valid act_info.json index."""
    t = _orig(arch)
    return {
        name: (funcs if name == _ACT_SET else (funcs - _ACT_SHARED))
        for name, funcs in t.items()
    }


def _build_program(gbar: float):
    orig_tables = bacc.get_activation_tables
    bacc.get_activation_tables = _patched_tables
    try:
        return _build_program_inner(gbar)
    finally:
        bacc.get_activation_tables = orig_tables


def _build_program_inner(gbar: float):
    nc = bacc.Bacc(
        "TRN2", target_bir_lowering=False, debug=False, num_devices=NCORES
    )
    xt_d = nc.dram_tensor("xt_in", [128, RC], F16, kind="ExternalInput").ap()
    w_d = nc.dram_tensor("w_in", [128, RP], F16, kind="ExternalInput").ap()
    v_d = nc.dram_tensor("v_in", [128, RC], F16, kind="ExternalInput").ap()
    y_d = nc.dram_tensor("y_out", [B, CS], F32, kind="ExternalOutput").ap()

    with tile.TileContext(nc) as tc:
        with (
            tc.tile_pool(name="pers", bufs=1) as pool,
            tc.tile_pool(name="acc", bufs=1, space="PSUM") as pspool,
        ):
            eps = pool.tile([128, 1], F32)
            nc.vector.memset(eps[:], 1e-24)
            xT = pool.tile([128, RC], F16)
            Ax = pool.tile([128, RC], F16)
            Lr = pool.tile([128, RC], F16)
            E1 = pool.tile([128, RC], F16)
            Z = pool.tile([128, RC], F16)
            Z1h = pool.tile([128, RC], F16)
            A0 = pool.tile([128, RP], F16)
            A1 = pool.tile([128, RC], F16)
            A2 = pool.tile([128, RC], F16)
            W0 = pool.tile([128, RP], F16)
            W1 = pool.tile([128, RC], F16)
            W2 = pool.tile([128, RC], F16)
            vt = pool.tile([128, RC], F16)
            yneg = pool.tile([128, CS], F32)
            ysb = pool.tile([128, CS], F32)
            acc = pspool.tile([128, 2 * CS], F32)

            # bias chunk of A0: 0.5 on partition 0, zero elsewhere
            nc.vector.memset(A0[:, RC:RP], 0.0)
            nc.vector.memset(A0[0:1, RC:RP], 0.5)

            # Uneven column halves: the second (last) slice is small so the
            # trailing Exp -> A0 -> A1 -> A2 -> matmul chain is short.
            H = 640
            halves = [slice(0, H), slice(H, RC)]

            # Input DMA on two parallel streams (sync = fast HWDGE, gpsimd =
            # slower SWDGE), column-split to match the compute halves so the
            # first Ln slice starts as soon as its columns land.  w_in is
            # host-reordered to [bias chunk | x chunks] so each W piece is one
            # contiguous DMA.  ScalarE issues no input DMA: its queue starts
            # with the hoisted act-table load.
            nc.sync.dma_start(xT[:, 0:H], xt_d[:, 0:H])
            nc.gpsimd.dma_start(xT[:, H:RC], xt_d[:, H:RC])
            nc.sync.dma_start(W0[:, 0 : 128 + H], w_d[:, 0 : 128 + H])
            nc.gpsimd.dma_start(W0[:, 128 + H : RP], w_d[:, 128 + H : RP])
            nc.sync.dma_start(vt[:, 0:H], v_d[:, 0:H])
            nc.gpsimd.dma_start(vt[:, H:RC], v_d[:, H:RC])

            # x chain: Ax = x^2 on DVE; Lr = ln(4x^2+eps) = 2 ln(2|x|) on
            # ScalarE (the 4x folds into the Ln input scale); E1 =
            # (2|x|)^(gbar-1) on ScalarE.  Ln/Exp interleave freely -- both
            # live in the same act-table set.  The whole x -> A chain is
            # high-priority so the Tile scheduler cannot slot W-chain ops
            # (which wait on the late v DMA) ahead of it in the DVE FIFO.
            with tc.high_priority():
                for sl in halves:
                    nc.vector.tensor_mul(Ax[:, sl], xT[:, sl], xT[:, sl])
                for sl in halves:
                    nc.scalar.activation(
                        Lr[:, sl], Ax[:, sl], ACT.Ln, bias=eps[:], scale=4.0
                    )
                    nc.scalar.activation(
                        E1[:, sl], Lr[:, sl], ACT.Exp, scale=(gbar - 1.0) / 2.0
                    )
                # DVE: z = Lr/(2 ln2) ; (z-1)/2 = Lr/(4 ln2) - 1/2 ;
                # A0 = x*E1 ; A1 = A0*z ; A2 = A1*(z-1)/2.
                for sl in halves:
                    nc.vector.tensor_scalar(
                        Z[:, sl], Lr[:, sl], 1.0 / (2 * LN2), None, ALU.mult
                    )
                    nc.vector.tensor_scalar(
                        Z1h[:, sl], Lr[:, sl], 1.0 / (4 * LN2), -0.5,
                        ALU.mult, ALU.add,
                    )
                    nc.vector.tensor_mul(A0[:, sl], xT[:, sl], E1[:, sl])
                    nc.vector.tensor_mul(A1[:, sl], A0[:, sl], Z[:, sl])
                    nc.vector.tensor_mul(A2[:, sl], A1[:, sl], Z1h[:, sl])

            # W chain (normal priority: fills DVE gaps, waits on v)
            for sl in halves:
                nc.vector.tensor_mul(W1[:, sl], W0[:, 128 + sl.start : 128 + sl.stop], vt[:, sl])
                nc.vector.tensor_mul(W2[:, sl], W1[:, sl], vt[:, sl])

            # 25 matmuls, one PSUM accumulation group.
            # k=0: bias chunk + 8 x-chunks ; k=1,2: 8 x-chunks each.
            nc.tensor.matmul(
                acc[:], A0[:, RC:RP], W0[:, 0:128], start=True, stop=False
            )
            for k, (Ak, Wk, off) in enumerate(
                ((A0, W0, 128), (A1, W1, 0), (A2, W2, 0))
            ):
                for ch in range(NCH):
                    sl = slice(off + ch * 128, off + (ch + 1) * 128)
                    asl = slice(ch * 128, (ch + 1) * 128)
                    nc.tensor.matmul(
                        acc[:], Ak[:, asl], Wk[:, sl],
                        start=False,
                        stop=(k == 2 and ch == NCH - 1),
                    )

            # y = pos + (-neg); neg columns were pre-negated on host
            nc.scalar.copy(yneg[:], acc[:, CS : 2 * CS])
            nc.vector.tensor_add(ysb[:], acc[:, 0:CS], yneg[:])
            nc.sync.dma_start(y_d[0:64, :], ysb[0:64, :])
            nc.scalar.dma_start(y_d[64:128, :], ysb[64:128, :])

    nc.compile()
    return nc


def _shard_inputs(x, w_pos, w_neg, b_pos, b_neg, n_param, m9, gbar):
    """Per-core input maps: slicing, layout swizzles, dtype casts, and affine
    weight prep (w -> +-(w+m9)*2^(gbar-1), n -> n*2^-gbar - 1, bias fold)."""

    def swizzle(host):  # [nch*128, 128] -> [128, nch*128] device layout
        nch = host.shape[0] // 128
        return np.ascontiguousarray(
            host.reshape(nch, 128, 128).transpose(1, 0, 2).reshape(128, nch * 128)
        )

    # xT[p, 128*ch + b] = x[b, 128*ch + p]
    xT = np.ascontiguousarray(
        x.astype(np.float16).reshape(128, NCH, 128).transpose(2, 1, 0).reshape(128, RC)
    )

    # exact bias-row fold: contribution 0.5*(b+m9)*n[N_IN, c]  (A-col is 0.5)
    nb = n_param[N_IN, :].astype(np.float64)
    yb_pos = (b_pos.astype(np.float64) + m9) * nb[0::2]
    yb_neg = -(b_neg.astype(np.float64) + m9) * nb[1::2]

    scale_v = np.float64(2.0) ** (-gbar)
    in_maps = []
    for j in range(NCORES):
        cp = slice(CS * j, CS * (j + 1))
        # w_in layout: [bias chunk | x chunks] so each DMA piece is contiguous
        W_host = np.zeros((RP, 128), np.float16)
        W_host[0, 0:CS] = yb_pos[cp]
        W_host[0, CS:128] = yb_neg[cp]
        W_host[128 : 128 + N_IN, 0:CS] = w_pos[:, cp] + np.float32(m9)
        W_host[128 : 128 + N_IN, CS:128] = -(w_neg[:, cp] + np.float32(m9))
        V_host = np.empty((RC, 128), np.float16)
        V_host[:, 0:CS] = n_param[:N_IN, 2 * CS * j : 2 * CS * (j + 1) : 2] * scale_v - 1.0
        V_host[:, CS:128] = (
            n_param[:N_IN, 2 * CS * j + 1 : 2 * CS * (j + 1) : 2] * scale_v - 1.0
        )
        in_maps.append(
            {
                "xt_in": xT,
                "w_in": swizzle(W_host),
                "v_in": swizzle(V_host),
            }
        )
    return in_maps


def kernel(x, w_pos, w_neg, b_pos, b_neg, n_param, **run_kwargs):
    global LAST_RESULTS
    x = np.ascontiguousarray(np.asarray(x, np.float32))
    w_pos = np.asarray(w_pos, np.float32)
    w_neg = np.asarray(w_neg, np.float32)
    b_pos = np.asarray(b_pos, np.float32)
    b_neg = np.asarray(b_neg, np.float32)
    n_param = np.asarray(n_param, np.float32)

    max_w = float(max(w_pos.max(), w_neg.max(), b_pos.max(), b_neg.max()))
    m9 = max_w / 9.0
    gbar = float(
        0.5 * (np.log2(float(n_param.min())) + np.log2(float(n_param.max())))
    )

    nc = _build_program(gbar)
    in_maps = _shard_inputs(x, w_pos, w_neg, b_pos, b_neg, n_param, m9, gbar)
    res = run_bass_kernel_spmd(nc, in_maps, list(range(NCORES)), **run_kwargs)
    LAST_RESULTS = res
    return np.concatenate([res.results[j]["y_out"] for j in range(NCORES)], axis=1)


# revision 22
# speedup vs baseline: 1.2019x; 1.0444x over previous
"""MemristorDense forward on 8 Trainium2 NeuronCores.

Math
----
Reference computes, with R = n_in+1 rows (x plus a ones bias-row), C = 2*n_out
interleaved pos/neg columns:

    y = 0.5 * sum_r sign(x) * (W+m9) * exp(L[b,r] * log2(n[r,c]))

with L = ln(max(2|x|,1e-12)), m9 = max_w/9.  Write n = 2^gbar * (1+v)
(gbar = midrange of log2(n), |v| <~ 0.26) and z = log2(2|x|).  Then
exp(L*log2 n) = (2|x|)^gbar * (1+v)^z, and the binomial series
(1+v)^z = sum_k C(z,k) v^k turns the [B,R,C] elementwise-pow contraction
into K+1 TensorEngine matmuls:

    y = sum_k  A_k.T @ W_k,   A_0 = x*|x|^(gbar-1)   (2^(gbar-1) folded into W)
                              A_1 = A_0 * z,  A_2 = A_1 * (z-1)/2
                              W_k = W_0 * v^k,  W_0 = +-(w + m9) * 2^(gbar-1)

K=2 suffices: truncation + fp16 quantization land at ~3e-3 relative against
the 2e-2 gate.  The ones bias-row is removed from the series: its
contribution 0.5*(b+m9)*n[n_in,c] is b-independent and exact, computed on the
host and folded into a k=0-only contraction chunk whose A-column is 0.5.

Everything on device is fp16; accumulation is fp32 in PSUM.  ScalarE does
only Ln and Exp (one act-table set, load hoisted before data arrives);
Square runs on VectorE.  DMAs are partition-split across two issue engines
because DMA time is packet-bound (~15ns per partition-packet), and dummy
matmuls warm the PE clock gate before the real accumulation.

Sharding: tensor-parallel over output columns (64 pos + 64 neg per core),
A-side replicated -- no collectives, gather is a pure concat.

Device layout: tiles are [128, 8*128] with tile[p, 128*ch + c] =
host_row[128*ch + p, c]; x-side free index is (ch, b), W-side is (ch, c).
"""

import numpy as np

import concourse.bacc as bacc
import concourse.tile as tile
import concourse.mybir as mybir
from concourse.bass_utils import run_bass_kernel_spmd

F32 = mybir.dt.float32
F16 = mybir.dt.float16
ALU = mybir.AluOpType
ACT = mybir.ActivationFunctionType

NCORES = 8
B = 128
N_IN = 1024
N_OUT = 512
NCH = 8                 # full 128-row chunks of real x rows
RC = NCH * 128          # 1024 real contraction rows
RP = RC + 128           # + bias chunk (k=0 only)
CS = N_OUT // NCORES    # 64 output columns per core
LN2 = 0.6931471805599453
# Stashed by kernel() for the test harness (exec_time_ns, trace paths).
LAST_RESULTS = None

_ACT_SET = "natural_log_exp_and_others"
_ACT_SHARED = {
    ACT.Square, ACT.Ln, ACT.Exp, ACT.Copy, ACT.Identity, ACT.Abs, ACT.Sign,
    ACT.MemsetZero,
}


def _patched_tables(arch, _orig=bacc.get_activation_tables):
    """Steer the act-table-load pass to a single table set: every function we
    use (ln/exp/copy) lives in natural_log_exp_and_others, but the greedy
    per-instruction chooser would otherwise pick several sets (~1.3us
    ACT_TABLE_LOAD each on the critical ScalarE chain).  Set names and order
    are preserved so act_func_set_id stays a valid act_info.json index."""
    t = _orig(arch)
    return {
        name: (funcs if name == _ACT_SET else (funcs - _ACT_SHARED))
        for name, funcs in t.items()
    }


def _build_program(gbar: float):
    orig_tables = bacc.get_activation_tables
    bacc.get_activation_tables = _patched_tables
    try:
        return _build_program_inner(gbar)
    finally:
        bacc.get_activation_tables = orig_tables


def _build_program_inner(gbar: float):
    nc = bacc.Bacc(
        "TRN2", target_bir_lowering=False, debug=False, num_devices=NCORES
    )
    xt_d = nc.dram_tensor("xt_in", [128, RC], F16, kind="ExternalInput").ap()
    w_d = nc.dram_tensor("w_in", [128, RP], F16, kind="ExternalInput").ap()
    v_d = nc.dram_tensor("v_in", [128, RC], F16, kind="ExternalInput").ap()
    y_d = nc.dram_tensor("y_out", [B, CS], F32, kind="ExternalOutput").ap()

    with tile.TileContext(nc) as tc:
        with (
            tc.tile_pool(name="pers", bufs=1) as pool,
            tc.tile_pool(name="acc", bufs=1, space="PSUM") as pspool,
        ):
            eps = pool.tile([128, 1], F32)
            nc.vector.memset(eps[:], 1e-24)
            xT = pool.tile([128, RC], F16)
            Ax = pool.tile([128, RC], F16)
            Lr = pool.tile([128, RC], F16)
            E1 = pool.tile([128, RC], F16)
            Z = pool.tile([128, RC], F16)
            Z1h = pool.tile([128, RC], F16)
            A0 = pool.tile([128, RP], F16)
            A1 = pool.tile([128, RC], F16)
            A2 = pool.tile([128, RC], F16)
            W0 = pool.tile([128, RP], F16)
            W1 = pool.tile([128, RC], F16)
            W2 = pool.tile([128, RC], F16)
            vt = pool.tile([128, RC], F16)
            yneg = pool.tile([128, CS], F32)
            ysb = pool.tile([128, CS], F32)
            acc = pspool.tile([128, 2 * CS], F32)

            # bias chunk of A0: 0.5 on partition 0, zero elsewhere
            nc.vector.memset(A0[:, RC:RP], 0.0)
            nc.vector.memset(A0[0:1, RC:RP], 0.5)

            # Uneven column halves: the second (last) slice is small so the
            # trailing Exp -> A0 -> A1 -> A2 -> matmul chain is short.
            H = 640
            halves = [slice(0, H), slice(H, RC)]

            # Input DMA on two parallel streams (sync = fast HWDGE, gpsimd =
            # slower SWDGE), column-split to match the compute halves so the
            # first Ln slice starts as soon as its columns land.  w_in is
            # host-reordered to [bias chunk | x chunks] so each W piece is one
            # contiguous DMA.  ScalarE issues no input DMA: its queue starts
            # with the hoisted act-table load.
            nc.sync.dma_start(xT[:, 0:H], xt_d[:, 0:H])
            nc.gpsimd.dma_start(xT[:, H:RC], xt_d[:, H:RC])
            nc.sync.dma_start(W0[:, 0 : 128 + H], w_d[:, 0 : 128 + H])
            nc.gpsimd.dma_start(W0[:, 128 + H : RP], w_d[:, 128 + H : RP])
            nc.sync.dma_start(vt[:, 0:H], v_d[:, 0:H])
            nc.gpsimd.dma_start(vt[:, H:RC], v_d[:, H:RC])

            # x chain: Ax = x^2 on DVE; Lr = ln(4x^2+eps) = 2 ln(2|x|) on
            # ScalarE (the 4x folds into the Ln input scale); E1 =
            # (2|x|)^(gbar-1) on ScalarE.  Ln/Exp interleave freely -- both
            # live in the same act-table set.  The whole x -> A chain is
            # high-priority so the Tile scheduler cannot slot W-chain ops
            # (which wait on the late v DMA) ahead of it in the DVE FIFO.
            with tc.high_priority():
                for sl in halves:
                    nc.vector.tensor_mul(Ax[:, sl], xT[:, sl], xT[:, sl])
                for sl in halves:
                    nc.scalar.activation(
                        Lr[:, sl], Ax[:, sl], ACT.Ln, bias=eps[:], scale=4.0
                    )
                    nc.scalar.activation(
                        E1[:, sl], Lr[:, sl], ACT.Exp, scale=(gbar - 1.0) / 2.0
                    )
                # DVE: z = Lr/(2 ln2) ; (z-1)/2 = Lr/(4 ln2) - 1/2 ;
                # A0 = x*E1 ; A1 = A0*z ; A2 = A1*(z-1)/2.
                for sl in halves:
                    nc.vector.tensor_scalar(
                        Z[:, sl], Lr[:, sl], 1.0 / (2 * LN2), None, ALU.mult
                    )
                    nc.vector.tensor_scalar(
                        Z1h[:, sl], Lr[:, sl], 1.0 / (4 * LN2), -0.5,
                        ALU.mult, ALU.add,
                    )
                    nc.vector.tensor_mul(A0[:, sl], xT[:, sl], E1[:, sl])
                    nc.vector.tensor_mul(A1[:, sl], A0[:, sl], Z[:, sl])
                    nc.vector.tensor_mul(A2[:, sl], A1[:, sl], Z1h[:, sl])

            # W chain.  tile_wait_until pushes the scheduler's modelled ready
            # time past the A-chain ops: its DMA model is optimistic about v's
            # arrival, and without this it packs W1/W2 ahead of the A-chain in
            # the DVE FIFO, head-of-line-blocking the critical path.
            with tc.tile_wait_until(0.004):
                for sl in halves:
                    nc.vector.tensor_mul(
                        W1[:, sl], W0[:, 128 + sl.start : 128 + sl.stop], vt[:, sl]
                    )
                    nc.vector.tensor_mul(W2[:, sl], W1[:, sl], vt[:, sl])

            # 25 matmuls, one PSUM accumulation group.
            # k=0: bias chunk + 8 x-chunks ; k=1,2: 8 x-chunks each.
            nc.tensor.matmul(
                acc[:], A0[:, RC:RP], W0[:, 0:128], start=True, stop=False
            )
            for k, (Ak, Wk, off) in enumerate(
                ((A0, W0, 128), (A1, W1, 0), (A2, W2, 0))
            ):
                for ch in range(NCH):
                    sl = slice(off + ch * 128, off + (ch + 1) * 128)
                    asl = slice(ch * 128, (ch + 1) * 128)
                    nc.tensor.matmul(
                        acc[:], Ak[:, asl], Wk[:, sl],
                        start=False,
                        stop=(k == 2 and ch == NCH - 1),
                    )

            # y = pos + (-neg); neg columns were pre-negated on host
            nc.scalar.copy(yneg[:], acc[:, CS : 2 * CS])
            nc.vector.tensor_add(ysb[:], acc[:, 0:CS], yneg[:])
            nc.sync.dma_start(y_d[0:64, :], ysb[0:64, :])
            nc.scalar.dma_start(y_d[64:128, :], ysb[64:128, :])

    nc.compile()
    return nc


def _shard_inputs(x, w_pos, w_neg, b_pos, b_neg, n_param, m9, gbar):
    """Per-core input maps: slicing, layout swizzles, dtype casts, and affine
    weight prep (w -> +-(w+m9)*2^(gbar-1), n -> n*2^-gbar - 1, bias fold)."""

    def swizzle(host):  # [nch*128, 128] -> [128, nch*128] device layout
        nch = host.shape[0] // 128
        return np.ascontiguousarray(
            host.reshape(nch, 128, 128).transpose(1, 0, 2).reshape(128, nch * 128)
        )

    # xT[p, 128*ch + b] = x[b, 128*ch + p]
    xT = np.ascontiguousarray(
        x.astype(np.float16).reshape(128, NCH, 128).transpose(2, 1, 0).reshape(128, RC)
    )

    # exact bias-row fold: contribution 0.5*(b+m9)*n[N_IN, c]  (A-col is 0.5)
    nb = n_param[N_IN, :].astype(np.float64)
    yb_pos = (b_pos.astype(np.float64) + m9) * nb[0::2]
    yb_neg = -(b_neg.astype(np.float64) + m9) * nb[1::2]

    scale_v = np.float64(2.0) ** (-gbar)
    in_maps = []
    for j in range(NCORES):
        cp = slice(CS * j, CS * (j + 1))
        # w_in layout: [bias chunk | x chunks] so each DMA piece is contiguous
        W_host = np.zeros((RP, 128), np.float16)
        W_host[0, 0:CS] = yb_pos[cp]
        W_host[0, CS:128] = yb_neg[cp]
        W_host[128 : 128 + N_IN, 0:CS] = w_pos[:, cp] + np.float32(m9)
        W_host[128 : 128 + N_IN, CS:128] = -(w_neg[:, cp] + np.float32(m9))
        V_host = np.empty((RC, 128), np.float16)
        V_host[:, 0:CS] = n_param[:N_IN, 2 * CS * j : 2 * CS * (j + 1) : 2] * scale_v - 1.0
        V_host[:, CS:128] = (
            n_param[:N_IN, 2 * CS * j + 1 : 2 * CS * (j + 1) : 2] * scale_v - 1.0
        )
        in_maps.append(
            {
                "xt_in": xT,
                "w_in": swizzle(W_host),
                "v_in": swizzle(V_host),
            }
        )
    return in_maps


def kernel(x, w_pos, w_neg, b_pos, b_neg, n_param, **run_kwargs):
    global LAST_RESULTS
    x = np.ascontiguousarray(np.asarray(x, np.float32))
    w_pos = np.asarray(w_pos, np.float32)
    w_neg = np.asarray(w_neg, np.float32)
    b_pos = np.asarray(b_pos, np.float32)
    b_neg = np.asarray(b_neg, np.float32)
    n_param = np.asarray(n_param, np.float32)

    max_w = float(max(w_pos.max(), w_neg.max(), b_pos.max(), b_neg.max()))
    m9 = max_w / 9.0
    gbar = float(
        0.5 * (np.log2(float(n_param.min())) + np.log2(float(n_param.max())))
    )

    nc = _build_program(gbar)
    in_maps = _shard_inputs(x, w_pos, w_neg, b_pos, b_neg, n_param, m9, gbar)
    res = run_bass_kernel_spmd(nc, in_maps, list(range(NCORES)), **run_kwargs)
    LAST_RESULTS = res
    return np.concatenate([res.results[j]["y_out"] for j in range(NCORES)], axis=1)


# revision 23
# speedup vs baseline: 1.2646x; 1.0522x over previous
"""MemristorDense forward on 8 Trainium2 NeuronCores.

Math
----
Reference computes, with R = n_in+1 rows (x plus a ones bias-row), C = 2*n_out
interleaved pos/neg columns:

    y = 0.5 * sum_r sign(x) * (W+m9) * exp(L[b,r] * log2(n[r,c]))

with L = ln(max(2|x|,1e-12)), m9 = max_w/9.  Write n = 2^gbar * (1+v)
(gbar = midrange of log2(n), |v| <~ 0.26) and z = log2(2|x|).  Then
exp(L*log2 n) = (2|x|)^gbar * (1+v)^z, and the binomial series
(1+v)^z = sum_k C(z,k) v^k turns the [B,R,C] elementwise-pow contraction
into K+1 TensorEngine matmuls:

    y = sum_k  A_k.T @ W_k,   A_0 = x*|x|^(gbar-1)   (2^(gbar-1) folded into W)
                              A_1 = A_0 * z,  A_2 = A_1 * (z-1)/2
                              W_k = W_0 * v^k,  W_0 = +-(w + m9) * 2^(gbar-1)

K=2 suffices: truncation + fp16 quantization land at ~3e-3 relative against
the 2e-2 gate.  The ones bias-row is removed from the series: its
contribution 0.5*(b+m9)*n[n_in,c] is b-independent and exact, computed on the
host and folded into a k=0-only contraction chunk whose A-column is 0.5.

Everything on device is fp16; accumulation is fp32 in PSUM.  ScalarE does
only Ln and Exp (one act-table set, load hoisted before data arrives);
Square runs on VectorE.  DMAs are partition-split across two issue engines
because DMA time is packet-bound (~15ns per partition-packet), and dummy
matmuls warm the PE clock gate before the real accumulation.

Sharding: tensor-parallel over output columns (64 pos + 64 neg per core),
A-side replicated -- no collectives, gather is a pure concat.

Device layout: tiles are [128, 8*128] with tile[p, 128*ch + c] =
host_row[128*ch + p, c]; x-side free index is (ch, b), W-side is (ch, c).
"""

import numpy as np

import concourse.bacc as bacc
import concourse.tile as tile
import concourse.mybir as mybir
from concourse.bass_utils import run_bass_kernel_spmd

F32 = mybir.dt.float32
F16 = mybir.dt.float16
ALU = mybir.AluOpType
ACT = mybir.ActivationFunctionType

NCORES = 8
B = 128
N_IN = 1024
N_OUT = 512
NCH = 8                 # full 128-row chunks of real x rows
RC = NCH * 128          # 1024 real contraction rows
RP = RC + 128           # + bias chunk (k=0 only)
CS = N_OUT // NCORES    # 64 output columns per core
LN2 = 0.6931471805599453
# Stashed by kernel() for the test harness (exec_time_ns, trace paths).
LAST_RESULTS = None

_ACT_SET = "natural_log_exp_and_others"
_ACT_SHARED = {
    ACT.Square, ACT.Ln, ACT.Exp, ACT.Copy, ACT.Identity, ACT.Abs, ACT.Sign,
    ACT.MemsetZero,
}


def _patched_tables(arch, _orig=bacc.get_activation_tables):
    """Steer the act-table-load pass to a single table set: every function we
    use (ln/exp/copy) lives in natural_log_exp_and_others, but the greedy
    per-instruction chooser would otherwise pick several sets (~1.3us
    ACT_TABLE_LOAD each on the critical ScalarE chain).  Set names and order
    are preserved so act_func_set_id stays a valid act_info.json index."""
    t = _orig(arch)
    return {
        name: (funcs if name == _ACT_SET else (funcs - _ACT_SHARED))
        for name, funcs in t.items()
    }


def _build_program(gbar: float):
    orig_tables = bacc.get_activation_tables
    bacc.get_activation_tables = _patched_tables
    try:
        return _build_program_inner(gbar)
    finally:
        bacc.get_activation_tables = orig_tables


def _build_program_inner(gbar: float):
    nc = bacc.Bacc(
        "TRN2", target_bir_lowering=False, debug=False, num_devices=NCORES
    )
    xt_d = nc.dram_tensor("xt_in", [128, RC], F16, kind="ExternalInput").ap()
    w_d = nc.dram_tensor("w_in", [128, RP], F16, kind="ExternalInput").ap()
    v_d = nc.dram_tensor("v_in", [128, RC], F16, kind="ExternalInput").ap()
    y_d = nc.dram_tensor("y_out", [B, CS], F32, kind="ExternalOutput").ap()

    with tile.TileContext(nc) as tc:
        with (
            tc.tile_pool(name="pers", bufs=1) as pool,
            tc.tile_pool(name="acc", bufs=1, space="PSUM") as pspool,
        ):
            eps = pool.tile([128, 1], F32)
            nc.vector.memset(eps[:], 1e-24)
            xT = pool.tile([128, RC], F16)
            Ax = pool.tile([128, RC], F16)
            Lr = pool.tile([128, RC], F16)
            E1 = pool.tile([128, RC], F16)
            Z = pool.tile([128, RC], F16)
            Z1h = pool.tile([128, RC], F16)
            A0 = pool.tile([128, RP], F16)
            A1 = pool.tile([128, RC], F16)
            A2 = pool.tile([128, RC], F16)
            W0 = pool.tile([128, RP], F16)
            W1 = pool.tile([128, RC], F16)
            W2 = pool.tile([128, RC], F16)
            vt = pool.tile([128, RC], F16)
            yneg = pool.tile([128, CS], F32)
            ysb = pool.tile([128, CS], F32)
            acc = pspool.tile([128, 2 * CS], F32)

            # bias chunk of A0: 0.5 on partition 0, zero elsewhere
            nc.vector.memset(A0[:, RC:RP], 0.0)
            nc.vector.memset(A0[0:1, RC:RP], 0.5)

            # Even column halves aligned with the DMA split and chunk groups.
            H = 512
            sl_a = slice(0, H)
            sl_b = slice(H, RC)

            # Input DMA on two parallel streams (sync = fast HWDGE, gpsimd =
            # slower SWDGE), column-split to match the compute halves so the
            # first Ln slice starts as soon as its columns land.  w_in is
            # host-reordered to [bias chunk | x chunks] so each W piece is one
            # contiguous DMA.  ScalarE issues no input DMA: its queue starts
            # with the hoisted act-table load.
            nc.sync.dma_start(xT[:, sl_a], xt_d[:, sl_a])
            nc.gpsimd.dma_start(xT[:, sl_b], xt_d[:, sl_b])
            nc.sync.dma_start(W0[:, 0 : 128 + H], w_d[:, 0 : 128 + H])
            nc.gpsimd.dma_start(W0[:, 128 + H : RP], w_d[:, 128 + H : RP])
            nc.sync.dma_start(vt[:, sl_a], v_d[:, sl_a])
            nc.gpsimd.dma_start(vt[:, sl_b], v_d[:, sl_b])

            # x chain per half: Ax = x^2 on DVE; Lr = ln(4x^2+eps) =
            # 2 ln(2|x|) on ScalarE (4x folded into the Ln input scale);
            # E1 = (2|x|)^(gbar-1) on ScalarE (same act-table set as Ln).
            # DVE: z = Lr/(2 ln2) ; (z-1)/2 = Lr/(4 ln2) - 1/2 ;
            # A0 = x*E1 ; A1 = A0*z ; A2 = A1*(z-1)/2.
            # Staged tile_wait_until values order the DVE FIFO: slice-a chain
            # first, then slice-b, then the W chain (the scheduler's DMA model
            # is optimistic about v and would otherwise pack W1/W2 ahead of
            # the critical A path, head-of-line-blocking it).
            def x_chain(sl):
                nc.vector.tensor_mul(Ax[:, sl], xT[:, sl], xT[:, sl])
                nc.scalar.activation(
                    Lr[:, sl], Ax[:, sl], ACT.Ln, bias=eps[:], scale=4.0
                )
                nc.scalar.activation(
                    E1[:, sl], Lr[:, sl], ACT.Exp, scale=(gbar - 1.0) / 2.0
                )
                nc.vector.tensor_scalar(
                    Z[:, sl], Lr[:, sl], 1.0 / (2 * LN2), None, ALU.mult
                )
                nc.vector.tensor_scalar(
                    Z1h[:, sl], Lr[:, sl], 1.0 / (4 * LN2), -0.5,
                    ALU.mult, ALU.add,
                )
                nc.vector.tensor_mul(A0[:, sl], xT[:, sl], E1[:, sl])
                nc.vector.tensor_mul(A1[:, sl], A0[:, sl], Z[:, sl])
                nc.vector.tensor_mul(A2[:, sl], A1[:, sl], Z1h[:, sl])

            x_chain(sl_a)
            with tc.tile_wait_until(0.0035):
                x_chain(sl_b)

            with tc.tile_wait_until(0.0055):
                for sl in (sl_a, sl_b):
                    nc.vector.tensor_mul(
                        W1[:, sl], W0[:, 128 + sl.start : 128 + sl.stop], vt[:, sl]
                    )
                    nc.vector.tensor_mul(W2[:, sl], W1[:, sl], vt[:, sl])

            # 25 matmuls, one PSUM accumulation group.  PE executes its FIFO
            # in order, so group them [bias, k0a, k1a, k0b, k1b, k2a, k2b]:
            # the a-half matmuls stream while the b-half chain still computes,
            # and the W2-gated k2 groups come last.
            nc.tensor.matmul(
                acc[:], A0[:, RC:RP], W0[:, 0:128], start=True, stop=False
            )
            CHA = H // 128
            groups = [
                (A0, W0, 128, 0, CHA), (A1, W1, 0, 0, CHA),
                (A0, W0, 128, CHA, NCH), (A1, W1, 0, CHA, NCH),
                (A2, W2, 0, 0, CHA), (A2, W2, 0, CHA, NCH),
            ]
            last = groups[-1]
            for gi, (Ak, Wk, off, c0, c1) in enumerate(groups):
                for ch in range(c0, c1):
                    sl = slice(off + ch * 128, off + (ch + 1) * 128)
                    asl = slice(ch * 128, (ch + 1) * 128)
                    nc.tensor.matmul(
                        acc[:], Ak[:, asl], Wk[:, sl],
                        start=False,
                        stop=(gi == len(groups) - 1 and ch == c1 - 1),
                    )

            # y = pos + (-neg); neg columns were pre-negated on host
            nc.scalar.copy(yneg[:], acc[:, CS : 2 * CS])
            nc.vector.tensor_add(ysb[:], acc[:, 0:CS], yneg[:])
            nc.sync.dma_start(y_d[0:64, :], ysb[0:64, :])
            nc.scalar.dma_start(y_d[64:128, :], ysb[64:128, :])

    nc.compile()
    return nc


def _shard_inputs(x, w_pos, w_neg, b_pos, b_neg, n_param, m9, gbar):
    """Per-core input maps: slicing, layout swizzles, dtype casts, and affine
    weight prep (w -> +-(w+m9)*2^(gbar-1), n -> n*2^-gbar - 1, bias fold)."""

    def swizzle(host):  # [nch*128, 128] -> [128, nch*128] device layout
        nch = host.shape[0] // 128
        return np.ascontiguousarray(
            host.reshape(nch, 128, 128).transpose(1, 0, 2).reshape(128, nch * 128)
        )

    # xT[p, 128*ch + b] = x[b, 128*ch + p]
    xT = np.ascontiguousarray(
        x.astype(np.float16).reshape(128, NCH, 128).transpose(2, 1, 0).reshape(128, RC)
    )

    # exact bias-row fold: contribution 0.5*(b+m9)*n[N_IN, c]  (A-col is 0.5)
    nb = n_param[N_IN, :].astype(np.float64)
    yb_pos = (b_pos.astype(np.float64) + m9) * nb[0::2]
    yb_neg = -(b_neg.astype(np.float64) + m9) * nb[1::2]

    scale_v = np.float64(2.0) ** (-gbar)
    in_maps = []
    for j in range(NCORES):
        cp = slice(CS * j, CS * (j + 1))
        # w_in layout: [bias chunk | x chunks] so each DMA piece is contiguous
        W_host = np.zeros((RP, 128), np.float16)
        W_host[0, 0:CS] = yb_pos[cp]
        W_host[0, CS:128] = yb_neg[cp]
        W_host[128 : 128 + N_IN, 0:CS] = w_pos[:, cp] + np.float32(m9)
        W_host[128 : 128 + N_IN, CS:128] = -(w_neg[:, cp] + np.float32(m9))
        V_host = np.empty((RC, 128), np.float16)
        V_host[:, 0:CS] = n_param[:N_IN, 2 * CS * j : 2 * CS * (j + 1) : 2] * scale_v - 1.0
        V_host[:, CS:128] = (
            n_param[:N_IN, 2 * CS * j + 1 : 2 * CS * (j + 1) : 2] * scale_v - 1.0
        )
        in_maps.append(
            {
                "xt_in": xT,
                "w_in": swizzle(W_host),
                "v_in": swizzle(V_host),
            }
        )
    return in_maps


def kernel(x, w_pos, w_neg, b_pos, b_neg, n_param, **run_kwargs):
    global LAST_RESULTS
    x = np.ascontiguousarray(np.asarray(x, np.float32))
    w_pos = np.asarray(w_pos, np.float32)
    w_neg = np.asarray(w_neg, np.float32)
    b_pos = np.asarray(b_pos, np.float32)
    b_neg = np.asarray(b_neg, np.float32)
    n_param = np.asarray(n_param, np.float32)

    max_w = float(max(w_pos.max(), w_neg.max(), b_pos.max(), b_neg.max()))
    m9 = max_w / 9.0
    gbar = float(
        0.5 * (np.log2(float(n_param.min())) + np.log2(float(n_param.max())))
    )

    nc = _build_program(gbar)
    in_maps = _shard_inputs(x, w_pos, w_neg, b_pos, b_neg, n_param, m9, gbar)
    res = run_bass_kernel_spmd(nc, in_maps, list(range(NCORES)), **run_kwargs)
    LAST_RESULTS = res
    return np.concatenate([res.results[j]["y_out"] for j in range(NCORES)], axis=1)
